# revision 1
# baseline (speedup 1.0000x reference)
"""Sparse L1-distance attention (nn_L1AttnSparse) on 8 Trainium2 NeuronCores.

Sharding: dst tokens are split across the 8 cores (256 dst tokens each);
every core keeps the full k/v tables (8 MB each) in DRAM and uses the
custom SWDGE gather instruction (dma_gather / InstDMAGatherAnt) to pull
the 2 KB k/v rows for its edges.  Scores, softmax over the 32 slots and
the weighted v-sum run on DVE/ACT.  Batch index is folded into the gather
index (tables are [2*2048, 512]).
"""

import sys

sys.path.insert(0, "/opt/trn_rl_repo")

import numpy as np

import concourse.bass as bass
import concourse.tile as tile
from concourse import bacc, mybir
from concourse.bass_utils import run_bass_kernel_spmd

BS = 2
N_TOK = 2048
NH = 8
W = 64
S = 32  # dst_mxlen
HW = NH * W  # 512 floats per (b, tok) row
N_CORES = 8
DT = N_TOK // N_CORES  # dst tokens per core = 256
CHUNKS = DT // 128  # dst chunks of 128 per core = 2
SH = 4  # slot halves per chunk (gather granularity)
SLOTS_PER = S // SH  # 16
IDX_PER = 128 * SLOTS_PER  # 2048 indices per gather


def _wrap_idx(flat):
    """int16 index list -> [128, n/16] tile layout: idx i at [i%16, i//16],
    replicated down the 8 groups of 16 partitions."""
    n = flat.shape[0]
    w16 = np.zeros((16, n // 16), dtype=np.int16)
    w16[np.arange(n) % 16, np.arange(n) // 16] = flat
    return np.tile(w16, (8, 1))


def build_kernel():
    nc = bacc.Bacc(
        "TRN2", target_bir_lowering=False, debug=False, num_devices=N_CORES,
        dynamic_dma_scratch_size=16384 * 8,
    )
    f32 = mybir.dt.float32
    i16 = mybir.dt.int16

    kf = nc.dram_tensor("kf", [BS * N_TOK, HW], f32, kind="ExternalInput").ap()
    vf = nc.dram_tensor("vf", [BS * N_TOK, HW], f32, kind="ExternalInput").ap()
    qc = nc.dram_tensor("qc", [BS, CHUNKS, 128, HW], f32, kind="ExternalInput").ap()
    idx = nc.dram_tensor(
        "idx", [BS, CHUNKS, SH, 128, IDX_PER // 16], i16, kind="ExternalInput"
    ).ap()
    oc = nc.dram_tensor("oc", [BS, CHUNKS, 128, HW], f32, kind="ExternalOutput").ap()

    with tile.TileContext(nc) as tc:
        with (
            tc.tile_pool(name="big", bufs=4) as bigp,
            tc.tile_pool(name="small", bufs=3) as smp,
            tc.tile_pool(name="idxp", bufs=4) as idxp,
        ):
            for b in range(BS):
                for c in range(CHUNKS):
                    q_t = smp.tile([128, HW], f32, tag="q")
                    nc.sync.dma_start(out=q_t[:], in_=qc[b, c])
                    L = smp.tile([128, S * NH], f32, tag="L")
                    idx_ts = []
                    for sh in range(SH):
                        it = idxp.tile([128, IDX_PER // 16], i16, tag=f"idx{sh}")
                        nc.sync.dma_start(out=it[:], in_=idx[b, c, sh])
                        idx_ts.append(it)
                    for sh in range(SH):
                        kg = bigp.tile([128, SLOTS_PER, HW], f32, tag="g")
                        nc.gpsimd.dma_gather(
                            kg[:], kf, idx_ts[sh][:], IDX_PER, IDX_PER, HW,
                            queue_num=0,
                        )
                        # kg <- kg - q (broadcast q over the slot dim)
                        nc.vector.tensor_tensor(
                            out=kg[:],
                            in0=kg[:],
                            in1=q_t[:, None, :].to_broadcast([128, SLOTS_PER, HW]),
                            op=mybir.AluOpType.subtract,
                        )
                        # L[:, sh half] = sum_w |kg|   ([128, s*h])
                        nc.vector.tensor_reduce(
                            out=L[:, sh * SLOTS_PER * NH : (sh + 1) * SLOTS_PER * NH],
                            in_=kg[:].rearrange("p s (h w) -> p (s h) w", w=W),
                            axis=mybir.AxisListType.X,
                            op=mybir.AluOpType.add,
                            apply_absolute_value=True,
                        )
                    # --- softmax over s (strided views: L is [p, (s h)]) ---
                    Lv = L[:].rearrange("p (s h) -> p h s", h=NH)
                    Lmin = smp.tile([128, NH], f32, tag="lmin")
                    nc.vector.tensor_reduce(
                        out=Lmin[:], in_=Lv, axis=mybir.AxisListType.X,
                        op=mybir.AluOpType.min,
                    )
                    E = smp.tile([128, S * NH], f32, tag="E")
                    nc.vector.tensor_tensor(
                        out=E[:].rearrange("p (s h) -> p s h", h=NH),
                        in0=L[:].rearrange("p (s h) -> p s h", h=NH),
                        in1=Lmin[:, None, :].to_broadcast([128, S, NH]),
                        op=mybir.AluOpType.subtract,
                    )
                    nc.scalar.activation(
                        out=E[:], in_=E[:], func=mybir.ActivationFunctionType.Exp,
                        scale=-1.0 / np.sqrt(W),
                    )
                    den = smp.tile([128, NH], f32, tag="den")
                    nc.vector.tensor_reduce(
                        out=den[:],
                        in_=E[:].rearrange("p (s h) -> p h s", h=NH),
                        axis=mybir.AxisListType.X,
                        op=mybir.AluOpType.add,
                    )
                    rden = smp.tile([128, NH], f32, tag="rden")
                    nc.vector.reciprocal(rden[:], den[:])
                    Wt = smp.tile([128, S * NH], f32, tag="Wt")
                    nc.vector.tensor_tensor(
                        out=Wt[:].rearrange("p (s h) -> p s h", h=NH),
                        in0=E[:].rearrange("p (s h) -> p s h", h=NH),
                        in1=rden[:, None, :].to_broadcast([128, S, NH]),
                        op=mybir.AluOpType.mult,
                    )
                    # --- weighted v gather+sum ---
                    ot = None
                    for sh in range(SH):
                        vg = bigp.tile([128, SLOTS_PER, HW], f32, tag="g")
                        nc.gpsimd.dma_gather(
                            vg[:], vf, idx_ts[sh][:], IDX_PER, IDX_PER, HW,
                            queue_num=0,
                        )
                        wslice = Wt[:, sh * SLOTS_PER * NH : (sh + 1) * SLOTS_PER * NH]
                        nc.vector.tensor_tensor(
                            out=vg[:].rearrange("p s (h w) -> p s h w", w=W),
                            in0=vg[:].rearrange("p s (h w) -> p s h w", w=W),
                            in1=wslice.rearrange("p (s h) -> p s h", h=NH)[
                                :, :, :, None
                            ].to_broadcast([128, SLOTS_PER, NH, W]),
                            op=mybir.AluOpType.mult,
                        )
                        on = smp.tile([128, HW], f32, tag="on")
                        nc.vector.tensor_reduce(
                            out=on[:],
                            in_=vg[:].rearrange("p s hw -> p hw s"),
                            axis=mybir.AxisListType.X,
                            op=mybir.AluOpType.add,
                        )
                        if ot is None:
                            ot = on
                        else:
                            acc = smp.tile([128, HW], f32, tag="acc")
                            nc.vector.tensor_tensor(
                                out=acc[:], in0=ot[:], in1=on[:],
                                op=mybir.AluOpType.add,
                            )
                            ot = acc
                    nc.sync.dma_start(out=oc[b, c], in_=ot[:])
    nc.compile()
    return nc


_NC_CACHE = None
_LAST_IN_MAPS = None


def kernel(v, q, k, coo, dst_mxlen):
    global _NC_CACHE
    assert int(dst_mxlen) == S
    v = np.asarray(v, dtype=np.float32)
    q = np.asarray(q, dtype=np.float32)
    k = np.asarray(k, dtype=np.float32)
    coo = np.asarray(coo)

    # src table: srct[t, s] = src index of edge (dst=t, slot=s)
    srct = np.zeros((N_TOK, S), dtype=np.int64)
    srct[coo[:, 0], coo[:, 2]] = coo[:, 1]

    kf = k.reshape(BS * N_TOK, HW)
    vf = v.reshape(BS * N_TOK, HW)

    if _NC_CACHE is None:
        _NC_CACHE = build_kernel()
    nc = _NC_CACHE

    in_maps = []
    for core in range(N_CORES):
        lo = core * DT
        qc = q[:, lo : lo + DT].reshape(BS, CHUNKS, 128, HW)
        idx = np.zeros((BS, CHUNKS, SH, 128, IDX_PER // 16), dtype=np.int16)
        for b in range(BS):
            for c in range(CHUNKS):
                for sh in range(SH):
                    # index i = s_local*128 + p  ->  row b*2048 + srct[...]
                    sl = np.arange(SLOTS_PER) + sh * SLOTS_PER
                    flat = (
                        b * N_TOK
                        + srct[lo + c * 128 : lo + (c + 1) * 128, sl].T
                    ).reshape(-1).astype(np.int16)  # [s_local, p] -> flat
                    idx[b, c, sh] = _wrap_idx(flat)
        in_maps.append(
            {"kf": kf, "vf": vf, "qc": np.ascontiguousarray(qc), "idx": idx}
        )

    global _LAST_IN_MAPS
    _LAST_IN_MAPS = in_maps
    res = run_bass_kernel_spmd(nc, in_maps, list(range(N_CORES)))
    out = np.empty((BS, N_TOK, NH, W), dtype=np.float32)
    for core in range(N_CORES):
        lo = core * DT
        out[:, lo : lo + DT] = res.results[core]["oc"].reshape(BS, DT, NH, W)
    return out



# revision 8
# speedup vs baseline: 2.1239x; 2.1239x over previous
"""Sparse L1-distance attention (nn_L1AttnSparse) on 8 Trainium2 NeuronCores.

Sharding: dst tokens split across the 8 cores (256 dst each, x2 batches).
Per (batch, 128-dst chunk) iteration the 4096 edges (128 dst x 32 slots)
are processed as:

  - k rows gathered fp16 TRANSPOSED ([w-lane partitions, edge free], 8
    gathers of 512 edges) so the per-(slot,head) |q-k| sum over w runs on
    the PE array: stationary = |kT - qT| slot-plane [128 w-lanes, 128 dst],
    moving = an 8-col head mask; PSUM accumulates the 4 w-lane groups and
    scores land directly as [128 dst, (slot, head)].
  - softmax skips the max-subtraction (L = sum|q-k|/8 <= ~25 so exp(-L)
    stays inside fp32 range): just exp on ACT + sum + reciprocal.
  - v rows gathered fp16 untransposed in w-major row layout (4 gathers of
    1024 edges) so the weight broadcast keeps every operand innermost-
    packed and all elementwise work runs in the DVE 2x fp16 mode; the slot
    sum is an in-place pairwise tree.

k and q are pre-scaled by 1/8 (= 1/sqrt(64)) on the host.
"""

import sys

sys.path.insert(0, "/opt/trn_rl_repo")

import numpy as np

import concourse.bass as bass
import concourse.tile as tile
from concourse import bacc, mybir
from concourse.bass_utils import run_bass_kernel_spmd

BS = 2
N_TOK = 2048
NH = 8
W = 64
S = 32  # dst_mxlen
HW = NH * W  # 512 values per token row
N_CORES = 8
DT = N_TOK // N_CORES  # dst tokens per core = 256
CHUNKS = DT // 128  # dst chunks of 128 per core = 2
CQ = HW // 128  # 4 w-lane groups (transposed gather rows per partition)
KG = 8  # k gathers per chunk (512 edges each = 4 slots)
KS = S // KG  # 4 slots per k gather
KNI = 128 * KS  # 512
VG = 4  # v gathers per chunk (1024 edges each = 8 slots)
VS = S // VG  # 8 slots per v gather
VNI = 128 * VS  # 1024


def _wrap_idx(flat):
    """int16 index list -> [128, n/16] tile layout: idx i at [i%16, i//16],
    replicated down the 8 groups of 16 partitions."""
    n = flat.shape[0]
    w16 = np.zeros((16, n // 16), dtype=np.int16)
    w16[np.arange(n) % 16, np.arange(n) // 16] = flat
    return np.tile(w16, (8, 1))


def build_kernel():
    nc = bacc.Bacc(
        "TRN2", target_bir_lowering=False, debug=False, num_devices=N_CORES,
        dynamic_dma_scratch_size=16384 * 4,
    )
    f16 = mybir.dt.float16
    f32 = mybir.dt.float32
    i16 = mybir.dt.int16
    A = mybir.AluOpType

    kt = nc.dram_tensor("kt", [BS * N_TOK, HW], f16, kind="ExternalInput").ap()
    vt = nc.dram_tensor("vt", [BS * N_TOK, HW], f16, kind="ExternalInput").ap()
    qT = nc.dram_tensor("qT", [BS, CHUNKS, 128, HW], f16, kind="ExternalInput").ap()
    idx = nc.dram_tensor(
        "idx", [BS, CHUNKS, 128, S * 128 // 16], i16, kind="ExternalInput"
    ).ap()
    msk = nc.dram_tensor("msk", [128, CQ * NH], f16, kind="ExternalInput").ap()
    oc = nc.dram_tensor("oc", [BS, CHUNKS, 128, HW], f16, kind="ExternalOutput").ap()

    with tile.TileContext(nc) as tc:
        with (
            tc.tile_pool(name="kp", bufs=10) as kp,      # 0.5MB k gather tiles
            tc.tile_pool(name="vp", bufs=6) as vp,       # 1MB v gather tiles
            tc.tile_pool(name="sp", bufs=10) as sp,      # small tiles
            tc.tile_pool(name="ip", bufs=3) as ip,       # idx tiles
            tc.psum_pool(name="pp", bufs=2) as pp,       # scores
        ):
            msk_t = sp.tile([128, CQ * NH], f16, tag="msk", bufs=1)
            nc.sync.dma_start(out=msk_t[:], in_=msk)

            for b in range(BS):
                for c in range(CHUNKS):
                    q_t = sp.tile([128, HW], f16, tag="qT")
                    nc.sync.dma_start(out=q_t[:], in_=qT[b, c])
                    qv = q_t[:].rearrange("p (cq d) -> p cq d", d=128)[
                        :, :, None, :
                    ].to_broadcast([128, CQ, KS, 128])
                    it = ip.tile([128, S * 128 // 16], i16, tag="idx")
                    nc.sync.dma_start(out=it[:], in_=idx[b, c])
                    itv = it[:].rearrange("p (g n) -> p g n", n=KNI // 16)

                    psum = pp.tile([128, S * NH], f32, tag="L")
                    for g in range(KG):
                        kg = kp.tile([128, CQ, KNI], f16, tag="kg")
                        nc.gpsimd.dma_gather(
                            kg[:], kt, itv[:, g], KNI, KNI, HW,
                            transpose=True, queue_num=0,
                        )
                        kv4 = kg[:].rearrange("p cq (s d) -> p cq s d", d=128)
                        # D = k/8 - q/8 (in place on the gather tile)
                        nc.vector.tensor_tensor(
                            out=kv4, in0=kv4, in1=qv, op=A.subtract,
                        )
                        # |D| in place on the ACT engine
                        nc.scalar.activation(
                            out=kg[:], in_=kg[:],
                            func=mybir.ActivationFunctionType.Abs,
                        )
                        # PE: L[d, (s h)] += sum_w |D| via head-mask matmuls
                        for s in range(KS):
                            sl = (g * KS + s) * NH
                            for cq in range(CQ):
                                nc.tensor.matmul(
                                    psum[:, sl : sl + NH],
                                    kv4[:, cq, s, :],
                                    msk_t[:, cq * NH : (cq + 1) * NH],
                                    start=(cq == 0),
                                    stop=(cq == CQ - 1),
                                )

                    # softmax over s (no max-subtraction: L is in [0, ~25])
                    E = sp.tile([128, S * NH], f16, tag="E")
                    nc.scalar.activation(
                        out=E[:], in_=psum[:],
                        func=mybir.ActivationFunctionType.Exp, scale=-1.0,
                    )
                    den = sp.tile([128, NH], f32, tag="den")
                    nc.vector.tensor_reduce(
                        out=den[:],
                        in_=E[:].rearrange("p (s h) -> p h s", h=NH),
                        axis=mybir.AxisListType.X,
                        op=A.add,
                    )
                    rden = sp.tile([128, NH], f32, tag="rden")
                    nc.vector.reciprocal(rden[:], den[:])

                    ogs = []
                    for g in range(VG):
                        vg = vp.tile([128, VS, HW], f16, tag="vg")
                        nc.gpsimd.dma_gather(
                            vg[:], vt,
                            it[:].rearrange("p (g n) -> p g n", n=VNI // 16)[:, g],
                            VNI, VNI, HW, queue_num=0,
                        )
                        ev = E[:, g * VS * NH : (g + 1) * VS * NH].rearrange(
                            "p (s h) -> p s h", h=NH
                        )[:, :, None, :].to_broadcast([128, VS, W, NH])
                        # weighted products in place, then slot tree 8 -> 1
                        nc.vector.tensor_tensor(
                            out=vg[:].rearrange("p s (w h) -> p s w h", h=NH),
                            in0=vg[:].rearrange("p s (w h) -> p s w h", h=NH),
                            in1=ev,
                            op=A.mult,
                        )
                        for hw_ in (4, 2, 1):
                            nc.vector.tensor_tensor(
                                out=vg[:, 0:hw_],
                                in0=vg[:, 0:hw_],
                                in1=vg[:, hw_ : 2 * hw_],
                                op=A.add,
                            )
                        ogs.append(vg)
                    o01 = sp.tile([128, HW], f16, tag="o01")
                    nc.vector.tensor_tensor(
                        out=o01[:], in0=ogs[0][:, 0], in1=ogs[1][:, 0], op=A.add,
                    )
                    o23 = sp.tile([128, HW], f16, tag="o23")
                    nc.vector.tensor_tensor(
                        out=o23[:], in0=ogs[2][:, 0], in1=ogs[3][:, 0], op=A.add,
                    )
                    Ot = sp.tile([128, HW], f16, tag="O")
                    nc.vector.tensor_tensor(
                        out=Ot[:], in0=o01[:], in1=o23[:], op=A.add,
                    )
                    # normalize: out = O * (1/den), broadcast over w
                    ot = sp.tile([128, HW], f16, tag="oc")
                    nc.vector.scalar_tensor_tensor(
                        out=ot[:].rearrange("p (w h) -> p w h", h=NH),
                        in0=Ot[:].rearrange("p (w h) -> p w h", h=NH),
                        scalar=1.0,
                        in1=rden[:][:, None, :].to_broadcast([128, W, NH]),
                        op0=A.mult,
                        op1=A.mult,
                    )
                    nc.sync.dma_start(out=oc[b, c], in_=ot[:])
    nc.compile()
    return nc


_NC_CACHE = None


def kernel(v, q, k, coo, dst_mxlen):
    global _NC_CACHE
    assert int(dst_mxlen) == S
    v = np.asarray(v, dtype=np.float32)
    q = np.asarray(q, dtype=np.float32)
    k = np.asarray(k, dtype=np.float32)
    coo = np.asarray(coo)

    # src table: srct[t, s] = src index of edge (dst=t, slot=s)
    srct = np.zeros((N_TOK, S), dtype=np.int64)
    srct[coo[:, 0], coo[:, 2]] = coo[:, 1]

    scale = 1.0 / np.sqrt(W)
    kt = (k * scale).astype(np.float16).reshape(BS * N_TOK, HW)
    # v table in w-major row layout: row[(w, h)] = v[h, w]
    vt = np.ascontiguousarray(v.transpose(0, 1, 3, 2)).astype(np.float16)
    vt = vt.reshape(BS * N_TOK, HW)
    qs = (q * scale).astype(np.float16).reshape(BS, N_TOK, HW)

    # PE head masks: msk[p, cq*8 + h] = 1 iff h == 2*cq + (p >= 64)
    msk = np.zeros((128, CQ * NH), dtype=np.float16)
    for cq in range(CQ):
        msk[0:64, cq * NH + 2 * cq] = 1.0
        msk[64:128, cq * NH + 2 * cq + 1] = 1.0

    if _NC_CACHE is None:
        _NC_CACHE = build_kernel()
    nc = _NC_CACHE

    in_maps = []
    for core in range(N_CORES):
        lo = core * DT
        # qT[p, (cq, d)] = q_scaled[b, lo + c*128 + d, cq*128 + p]
        qT = np.empty((BS, CHUNKS, 128, HW), dtype=np.float16)
        for b in range(BS):
            for c in range(CHUNKS):
                blk = qs[b, lo + c * 128 : lo + (c + 1) * 128]  # [128 d, 512]
                qT[b, c] = (
                    blk.reshape(128, CQ, 128).transpose(2, 1, 0).reshape(128, HW)
                )
        # edge i = s*128 + d; wrapped per 512-idx granule, concatenated
        idx = np.zeros((BS, CHUNKS, 128, S * 128 // 16), dtype=np.int16)
        for b in range(BS):
            for c in range(CHUNKS):
                flat = (
                    b * N_TOK + srct[lo + c * 128 : lo + (c + 1) * 128, :].T
                ).reshape(-1).astype(np.int16)  # [(s), (d)] flattened
                for g in range(KG):
                    idx[b, c, :, g * (KNI // 16) : (g + 1) * (KNI // 16)] = (
                        _wrap_idx(flat[g * KNI : (g + 1) * KNI])
                    )
        in_maps.append(
            {"kt": kt, "vt": vt, "qT": qT, "idx": idx, "msk": msk}
        )

    res = run_bass_kernel_spmd(nc, in_maps, list(range(N_CORES)))
    out = np.empty((BS, N_TOK, NH, W), dtype=np.float32)
    for core in range(N_CORES):
        lo = core * DT
        o = res.results[core]["oc"].astype(np.float32)  # [BS, CHUNKS, 128, (w h)]
        o = o.reshape(BS, CHUNKS, 128, W, NH).transpose(0, 1, 2, 4, 3)
        out[:, lo : lo + DT] = o.reshape(BS, DT, NH, W)
    return out


# revision 10
# speedup vs baseline: 2.3414x; 1.1024x over previous
"""Sparse L1-distance attention (nn_L1AttnSparse) on 8 Trainium2 NeuronCores.

Sharding: dst tokens split across the 8 cores (256 dst each, x2 batches).
Per (batch, 128-dst chunk) iteration the 4096 edges (128 dst x 32 slots)
are processed as:

  - k rows gathered fp16 TRANSPOSED ([w-lane partitions, edge free], 8
    gathers of 512 edges) so the per-(slot,head) |q-k| sum over w runs on
    the PE array: stationary = |kT - qT| slot-plane [128 w-lanes, 128 dst],
    moving = an 8-col head mask; PSUM accumulates the 4 w-lane groups and
    scores land directly as [128 dst, (slot, head)].
  - softmax skips the max-subtraction (L = sum|q-k|/8 <= ~25 so exp(-L)
    stays inside fp32 range): just exp on ACT + sum + reciprocal.
  - v rows gathered fp16 untransposed in w-major row layout (4 gathers of
    1024 edges) so the weight broadcast keeps every operand innermost-
    packed and all elementwise work runs in the DVE 2x fp16 mode; the slot
    sum is an in-place pairwise tree.

k and q are pre-scaled by 1/8 (= 1/sqrt(64)) on the host.
"""

import sys

sys.path.insert(0, "/opt/trn_rl_repo")

import numpy as np

import concourse.bass as bass
import concourse.tile as tile
from concourse import bacc, mybir
from concourse.bass_utils import run_bass_kernel_spmd

BS = 2
N_TOK = 2048
NH = 8
W = 64
S = 32  # dst_mxlen
HW = NH * W  # 512 values per token row
N_CORES = 8
DT = N_TOK // N_CORES  # dst tokens per core = 256
CHUNKS = DT // 128  # dst chunks of 128 per core = 2
CQ = HW // 128  # 4 w-lane groups (transposed gather rows per partition)
KG = 8  # k gathers per chunk (512 edges each = 4 slots)
KS = S // KG  # 4 slots per k gather
KNI = 128 * KS  # 512
VG = 4  # v gathers per chunk (1024 edges each = 8 slots)
VS = S // VG  # 8 slots per v gather
VNI = 128 * VS  # 1024


def _wrap_idx(flat):
    """int16 index list -> [128, n/16] tile layout: idx i at [i%16, i//16],
    replicated down the 8 groups of 16 partitions."""
    n = flat.shape[0]
    w16 = np.zeros((16, n // 16), dtype=np.int16)
    w16[np.arange(n) % 16, np.arange(n) // 16] = flat
    return np.tile(w16, (8, 1))


def build_kernel():
    nc = bacc.Bacc(
        "TRN2", target_bir_lowering=False, debug=False, num_devices=N_CORES,
        dynamic_dma_scratch_size=16384 * 4,
    )
    f16 = mybir.dt.float16
    f32 = mybir.dt.float32
    i16 = mybir.dt.int16
    A = mybir.AluOpType

    kt = nc.dram_tensor("kt", [BS * N_TOK, HW], f16, kind="ExternalInput").ap()
    vt = nc.dram_tensor("vt", [BS * N_TOK, HW], f16, kind="ExternalInput").ap()
    qT = nc.dram_tensor("qT", [BS, CHUNKS, 128, HW], f16, kind="ExternalInput").ap()
    idx = nc.dram_tensor(
        "idx", [BS, CHUNKS, 128, S * 128 // 16], i16, kind="ExternalInput"
    ).ap()
    msk = nc.dram_tensor("msk", [128, CQ * NH], f16, kind="ExternalInput").ap()
    oc = nc.dram_tensor("oc", [BS, CHUNKS, 128, HW], f16, kind="ExternalOutput").ap()

    with tile.TileContext(nc) as tc:
        with (
            tc.tile_pool(name="kp", bufs=10) as kp,      # 0.5MB k gather tiles
            tc.tile_pool(name="vp", bufs=6) as vp,       # 1MB v gather tiles
            tc.tile_pool(name="sp", bufs=10) as sp,      # small tiles
            tc.tile_pool(name="ip", bufs=3) as ip,       # idx tiles
            tc.psum_pool(name="pp", bufs=2) as pp,       # scores
        ):
            msk_t = sp.tile([128, CQ * NH], f16, tag="msk", bufs=1)
            nc.sync.dma_start(out=msk_t[:], in_=msk)

            for b in range(BS):
                for c in range(CHUNKS):
                    q_t = sp.tile([128, HW], f16, tag="qT")
                    nc.sync.dma_start(out=q_t[:], in_=qT[b, c])
                    qv = q_t[:].rearrange("p (cq d) -> p cq d", d=128)[
                        :, :, None, :
                    ].to_broadcast([128, CQ, KS, 128])
                    it = ip.tile([128, S * 128 // 16], i16, tag="idx")
                    nc.sync.dma_start(out=it[:], in_=idx[b, c])
                    itv = it[:].rearrange("p (g n) -> p g n", n=KNI // 16)

                    psum = pp.tile([128, S * NH], f32, tag="L")
                    E = sp.tile([128, S * NH], f16, tag="E")

                    def k_granule(g):
                        kg = kp.tile([128, CQ, KNI], f16, tag="kg")
                        nc.gpsimd.dma_gather(
                            kg[:], kt, itv[:, g], KNI, KNI, HW,
                            transpose=True, queue_num=0,
                        )
                        kv4 = kg[:].rearrange("p cq (s d) -> p cq s d", d=128)
                        # D = k/8 - q/8 (in place on the gather tile)
                        nc.vector.tensor_tensor(
                            out=kv4, in0=kv4, in1=qv, op=A.subtract,
                        )
                        # |D| in place on the ACT engine
                        nc.scalar.activation(
                            out=kg[:], in_=kg[:],
                            func=mybir.ActivationFunctionType.Abs,
                        )
                        # PE: L[d, (s h)] += sum_w |D| via head-mask matmuls
                        for s in range(KS):
                            sl = (g * KS + s) * NH
                            for cq in range(CQ):
                                nc.tensor.matmul(
                                    psum[:, sl : sl + NH],
                                    kv4[:, cq, s, :],
                                    msk_t[:, cq * NH : (cq + 1) * NH],
                                    start=(cq == 0),
                                    stop=(cq == CQ - 1),
                                )
                        # early per-granule exp so the v side can start
                        # before the remaining score granules finish
                        gs = g * KS * NH
                        nc.scalar.activation(
                            out=E[:, gs : gs + KS * NH],
                            in_=psum[:, gs : gs + KS * NH],
                            func=mybir.ActivationFunctionType.Exp, scale=-1.0,
                        )

                    ogs = []

                    def v_granule(g):
                        vg = vp.tile([128, VS, HW], f16, tag="vg")
                        nc.gpsimd.dma_gather(
                            vg[:], vt,
                            it[:].rearrange("p (g n) -> p g n", n=VNI // 16)[:, g],
                            VNI, VNI, HW, queue_num=0,
                        )
                        ev = E[:, g * VS * NH : (g + 1) * VS * NH].rearrange(
                            "p (s h) -> p s h", h=NH
                        )[:, :, None, :].to_broadcast([128, VS, W, NH])
                        # weighted products in place, then slot tree 8 -> 1
                        nc.vector.tensor_tensor(
                            out=vg[:].rearrange("p s (w h) -> p s w h", h=NH),
                            in0=vg[:].rearrange("p s (w h) -> p s w h", h=NH),
                            in1=ev,
                            op=A.mult,
                        )
                        for hw_ in (4, 2, 1):
                            nc.vector.tensor_tensor(
                                out=vg[:, 0:hw_],
                                in0=vg[:, 0:hw_],
                                in1=vg[:, hw_ : 2 * hw_],
                                op=A.add,
                            )
                        ogs.append(vg)

                    # interleave so the DMA feed matches consumption order:
                    # v slot-group g only needs score granules 2g, 2g+1
                    for g in range(VG):
                        k_granule(2 * g)
                        k_granule(2 * g + 1)
                        v_granule(g)

                    # normalizer (needs all granules; applied at the end)
                    den = sp.tile([128, NH], f32, tag="den")
                    nc.vector.tensor_reduce(
                        out=den[:],
                        in_=E[:].rearrange("p (s h) -> p h s", h=NH),
                        axis=mybir.AxisListType.X,
                        op=A.add,
                    )
                    rden = sp.tile([128, NH], f32, tag="rden")
                    nc.vector.reciprocal(rden[:], den[:])
                    rh = sp.tile([128, NH], f16, tag="rh")
                    nc.scalar.activation(
                        out=rh[:], in_=rden[:],
                        func=mybir.ActivationFunctionType.Copy,
                    )
                    o01 = sp.tile([128, HW], f16, tag="o01")
                    nc.vector.tensor_tensor(
                        out=o01[:], in0=ogs[0][:, 0], in1=ogs[1][:, 0], op=A.add,
                    )
                    o23 = sp.tile([128, HW], f16, tag="o23")
                    nc.vector.tensor_tensor(
                        out=o23[:], in0=ogs[2][:, 0], in1=ogs[3][:, 0], op=A.add,
                    )
                    Ot = sp.tile([128, HW], f16, tag="O")
                    nc.vector.tensor_tensor(
                        out=Ot[:], in0=o01[:], in1=o23[:], op=A.add,
                    )
                    # normalize: out = O * (1/den), broadcast over w
                    ot = sp.tile([128, HW], f16, tag="oc")
                    nc.vector.tensor_tensor(
                        out=ot[:].rearrange("p (w h) -> p w h", h=NH),
                        in0=Ot[:].rearrange("p (w h) -> p w h", h=NH),
                        in1=rh[:][:, None, :].to_broadcast([128, W, NH]),
                        op=A.mult,
                    )
                    nc.sync.dma_start(out=oc[b, c], in_=ot[:])
    nc.compile()
    return nc


_NC_CACHE = None


def kernel(v, q, k, coo, dst_mxlen):
    global _NC_CACHE
    assert int(dst_mxlen) == S
    v = np.asarray(v, dtype=np.float32)
    q = np.asarray(q, dtype=np.float32)
    k = np.asarray(k, dtype=np.float32)
    coo = np.asarray(coo)

    # src table: srct[t, s] = src index of edge (dst=t, slot=s)
    srct = np.zeros((N_TOK, S), dtype=np.int64)
    srct[coo[:, 0], coo[:, 2]] = coo[:, 1]

    scale = 1.0 / np.sqrt(W)
    kt = (k * scale).astype(np.float16).reshape(BS * N_TOK, HW)
    # v table in w-major row layout: row[(w, h)] = v[h, w]
    vt = np.ascontiguousarray(v.transpose(0, 1, 3, 2)).astype(np.float16)
    vt = vt.reshape(BS * N_TOK, HW)
    qs = (q * scale).astype(np.float16).reshape(BS, N_TOK, HW)

    # PE head masks: msk[p, cq*8 + h] = 1 iff h == 2*cq + (p >= 64)
    msk = np.zeros((128, CQ * NH), dtype=np.float16)
    for cq in range(CQ):
        msk[0:64, cq * NH + 2 * cq] = 1.0
        msk[64:128, cq * NH + 2 * cq + 1] = 1.0

    if _NC_CACHE is None:
        _NC_CACHE = build_kernel()
    nc = _NC_CACHE

    in_maps = []
    for core in range(N_CORES):
        lo = core * DT
        # qT[p, (cq, d)] = q_scaled[b, lo + c*128 + d, cq*128 + p]
        qT = np.empty((BS, CHUNKS, 128, HW), dtype=np.float16)
        for b in range(BS):
            for c in range(CHUNKS):
                blk = qs[b, lo + c * 128 : lo + (c + 1) * 128]  # [128 d, 512]
                qT[b, c] = (
                    blk.reshape(128, CQ, 128).transpose(2, 1, 0).reshape(128, HW)
                )
        # edge i = s*128 + d; wrapped per 512-idx granule, concatenated
        idx = np.zeros((BS, CHUNKS, 128, S * 128 // 16), dtype=np.int16)
        for b in range(BS):
            for c in range(CHUNKS):
                flat = (
                    b * N_TOK + srct[lo + c * 128 : lo + (c + 1) * 128, :].T
                ).reshape(-1).astype(np.int16)  # [(s), (d)] flattened
                for g in range(KG):
                    idx[b, c, :, g * (KNI // 16) : (g + 1) * (KNI // 16)] = (
                        _wrap_idx(flat[g * KNI : (g + 1) * KNI])
                    )
        in_maps.append(
            {"kt": kt, "vt": vt, "qT": qT, "idx": idx, "msk": msk}
        )

    res = run_bass_kernel_spmd(nc, in_maps, list(range(N_CORES)))
    out = np.empty((BS, N_TOK, NH, W), dtype=np.float32)
    for core in range(N_CORES):
        lo = core * DT
        o = res.results[core]["oc"].astype(np.float32)  # [BS, CHUNKS, 128, (w h)]
        o = o.reshape(BS, CHUNKS, 128, W, NH).transpose(0, 1, 2, 4, 3)
        out[:, lo : lo + DT] = o.reshape(BS, DT, NH, W)
    return out


# revision 15
# speedup vs baseline: 2.5911x; 1.1066x over previous
"""Sparse L1-distance attention (nn_L1AttnSparse) on 8 Trainium2 NeuronCores.

Sharding: dst tokens split across the 8 cores (256 dst each, x2 batches).
Per (batch, 128-dst chunk) iteration the 4096 edges (128 dst x 32 slots)
are processed as:

  - k rows gathered fp16 TRANSPOSED ([w-lane partitions, edge free], 8
    gathers of 512 edges) so the per-(slot,head) |q-k| sum over w runs on
    the PE array: stationary = |kT - qT| slot-plane [128 w-lanes, 128 dst],
    moving = an 8-col head mask; PSUM accumulates the 4 w-lane groups and
    scores land directly as [128 dst, (slot, head)].
  - softmax skips the max-subtraction (L = sum|q-k|/8 <= ~25 so exp(-L)
    stays inside fp32 range): just exp on ACT + sum + reciprocal.
  - v rows gathered fp16 untransposed in w-major row layout (4 gathers of
    1024 edges) so the weight broadcast keeps every operand innermost-
    packed and all elementwise work runs in the DVE 2x fp16 mode; the slot
    sum is an in-place pairwise tree.

k and q are pre-scaled by 1/8 (= 1/sqrt(64)) on the host.
"""

import sys

sys.path.insert(0, "/opt/trn_rl_repo")

import numpy as np

import concourse.bass as bass
import concourse.tile as tile
from concourse import bacc, mybir
from concourse.bass_utils import run_bass_kernel_spmd

BS = 2
N_TOK = 2048
NH = 8
W = 64
S = 32  # dst_mxlen
HW = NH * W  # 512 values per token row
N_CORES = 8
DT = N_TOK // N_CORES  # dst tokens per core = 256
CHUNKS = DT // 128  # dst chunks of 128 per core = 2
CQ = HW // 128  # 4 w-lane groups (transposed gather rows per partition)
KG = 8  # k gathers per chunk (512 edges each = 4 slots)
KS = S // KG  # 4 slots per k gather
KNI = 128 * KS  # 512
VG = 4  # v gathers per chunk (1024 edges each = 8 slots)
VS = S // VG  # 8 slots per v gather
VNI = 128 * VS  # 1024


def _wrap_idx(flat):
    """int16 index list -> [128, n/16] tile layout: idx i at [i%16, i//16],
    replicated down the 8 groups of 16 partitions."""
    n = flat.shape[0]
    w16 = np.zeros((16, n // 16), dtype=np.int16)
    w16[np.arange(n) % 16, np.arange(n) // 16] = flat
    return np.tile(w16, (8, 1))


def build_kernel():
    nc = bacc.Bacc(
        "TRN2", target_bir_lowering=False, debug=False, num_devices=N_CORES,
        dynamic_dma_scratch_size=16384 * 4,
    )
    f16 = mybir.dt.float16
    f32 = mybir.dt.float32
    i16 = mybir.dt.int16
    A = mybir.AluOpType

    kt = nc.dram_tensor("kt", [BS * N_TOK, HW], f16, kind="ExternalInput").ap()
    vt = nc.dram_tensor("vt", [BS * N_TOK, HW], f16, kind="ExternalInput").ap()
    qT = nc.dram_tensor("qT", [BS, CHUNKS, 128, HW], f16, kind="ExternalInput").ap()
    idx = nc.dram_tensor(
        "idx", [BS, CHUNKS, 128, S * 128 // 16], i16, kind="ExternalInput"
    ).ap()
    msk = nc.dram_tensor("msk", [128, CQ * NH], f16, kind="ExternalInput").ap()
    idn = nc.dram_tensor("idn", [128, 128], f16, kind="ExternalInput").ap()
    oc = nc.dram_tensor("oc", [BS, CHUNKS, 128, HW], f16, kind="ExternalOutput").ap()

    with tile.TileContext(nc) as tc:
        with (
            tc.tile_pool(name="kp", bufs=10) as kp,      # 0.5MB k gather tiles
            tc.tile_pool(name="vp", bufs=6) as vp,       # 1MB v gather tiles
            tc.tile_pool(name="sp", bufs=10) as sp,      # small tiles
            tc.tile_pool(name="ip", bufs=3) as ip,       # idx tiles
            tc.psum_pool(name="pp", bufs=2) as pp,       # scores
        ):
            msk_t = sp.tile([128, CQ * NH], f16, tag="msk", bufs=1)
            nc.sync.dma_start(out=msk_t[:], in_=msk)
            id_t = sp.tile([128, 128], f16, tag="idn", bufs=1)
            nc.sync.dma_start(out=id_t[:], in_=idn)

            for b in range(BS):
                for c in range(CHUNKS):
                    q_t = sp.tile([128, HW], f16, tag="qT")
                    nc.sync.dma_start(out=q_t[:], in_=qT[b, c])
                    qv = q_t[:].rearrange("p (cq d) -> p cq d", d=128)[
                        :, :, None, :
                    ].to_broadcast([128, CQ, KS, 128])
                    it = ip.tile([128, S * 128 // 16], i16, tag="idx")
                    nc.sync.dma_start(out=it[:], in_=idx[b, c])
                    itv = it[:].rearrange("p (g n) -> p g n", n=KNI // 16)

                    psum = pp.tile([128, S * NH], f32, tag="L")
                    E = sp.tile([128, S * NH], f16, tag="E")

                    def k_granule(g):
                        kg = kp.tile([128, CQ, KNI], f16, tag="kg")
                        nc.gpsimd.dma_gather(
                            kg[:], kt, itv[:, g], KNI, KNI, HW,
                            transpose=True, queue_num=0,
                        )
                        kv4 = kg[:].rearrange("p cq (s d) -> p cq s d", d=128)
                        # D = k/8 - q/8 (in place on the gather tile)
                        nc.vector.tensor_tensor(
                            out=kv4, in0=kv4, in1=qv, op=A.subtract,
                        )
                        # |D| in place on the ACT engine
                        nc.scalar.activation(
                            out=kg[:], in_=kg[:],
                            func=mybir.ActivationFunctionType.Abs,
                        )
                        # PE: L[d, (s h)] += sum_w |D| via head-mask matmuls
                        for s in range(KS):
                            sl = (g * KS + s) * NH
                            for cq in range(CQ):
                                nc.tensor.matmul(
                                    psum[:, sl : sl + NH],
                                    kv4[:, cq, s, :],
                                    msk_t[:, cq * NH : (cq + 1) * NH],
                                    start=(cq == 0),
                                    stop=(cq == CQ - 1),
                                )
                        # early per-granule exp so the v side can start
                        # before the remaining score granules finish
                        gs = g * KS * NH
                        nc.scalar.activation(
                            out=E[:, gs : gs + KS * NH],
                            in_=psum[:, gs : gs + KS * NH],
                            func=mybir.ActivationFunctionType.Exp, scale=-1.0,
                        )

                    psum_o = pp.tile([128, HW], f32, tag="O")

                    def v_granule(g):
                        vg = vp.tile([128, VS, HW], f16, tag="vg")
                        nc.gpsimd.dma_gather(
                            vg[:], vt,
                            it[:].rearrange("p (g n) -> p g n", n=VNI // 16)[:, g],
                            VNI, VNI, HW, queue_num=0,
                        )
                        ev = E[:, g * VS * NH : (g + 1) * VS * NH].rearrange(
                            "p (s h) -> p s h", h=NH
                        )[:, :, None, :].to_broadcast([128, VS, W, NH])
                        # weighted products in place
                        nc.vector.tensor_tensor(
                            out=vg[:].rearrange("p s (w h) -> p s w h", h=NH),
                            in0=vg[:].rearrange("p s (w h) -> p s w h", h=NH),
                            in1=ev,
                            op=A.mult,
                        )
                        # slot sum on PE: psum_o += I @ P_s
                        for s in range(VS):
                            nc.tensor.matmul(
                                psum_o[:],
                                id_t[:],
                                vg[:, s],
                                start=(g == 0 and s == 0),
                                stop=(g == VG - 1 and s == VS - 1),
                                skip_group_check=True,
                            )

                    # interleave so the DMA feed matches consumption order:
                    # v slot-group g only needs score granules 2g, 2g+1
                    for g in range(VG):
                        k_granule(2 * g)
                        k_granule(2 * g + 1)
                        v_granule(g)

                    # normalizer (needs all granules; applied at the end)
                    den = sp.tile([128, NH], f32, tag="den")
                    nc.vector.tensor_reduce(
                        out=den[:],
                        in_=E[:].rearrange("p (s h) -> p h s", h=NH),
                        axis=mybir.AxisListType.X,
                        op=A.add,
                    )
                    rden = sp.tile([128, NH], f32, tag="rden")
                    nc.vector.reciprocal(rden[:], den[:])
                    # normalize: out = psum_o * (1/den), broadcast over w
                    ot = sp.tile([128, HW], f16, tag="oc")
                    nc.vector.tensor_tensor(
                        out=ot[:].rearrange("p (w h) -> p w h", h=NH),
                        in0=psum_o[:].rearrange("p (w h) -> p w h", h=NH),
                        in1=rden[:][:, None, :].to_broadcast([128, W, NH]),
                        op=A.mult,
                    )
                    nc.sync.dma_start(out=oc[b, c], in_=ot[:])
    nc.compile()
    return nc


_NC_CACHE = None


def kernel(v, q, k, coo, dst_mxlen):
    global _NC_CACHE
    assert int(dst_mxlen) == S
    v = np.asarray(v, dtype=np.float32)
    q = np.asarray(q, dtype=np.float32)
    k = np.asarray(k, dtype=np.float32)
    coo = np.asarray(coo)

    # src table: srct[t, s] = src index of edge (dst=t, slot=s)
    srct = np.zeros((N_TOK, S), dtype=np.int64)
    srct[coo[:, 0], coo[:, 2]] = coo[:, 1]

    scale = 1.0 / np.sqrt(W)
    kt = (k * scale).astype(np.float16).reshape(BS * N_TOK, HW)
    # v table in w-major row layout: row[(w, h)] = v[h, w]
    vt = np.ascontiguousarray(v.transpose(0, 1, 3, 2)).astype(np.float16)
    vt = vt.reshape(BS * N_TOK, HW)
    qs = (q * scale).astype(np.float16).reshape(BS, N_TOK, HW)

    # PE head masks: msk[p, cq*8 + h] = 1 iff h == 2*cq + (p >= 64)
    msk = np.zeros((128, CQ * NH), dtype=np.float16)
    for cq in range(CQ):
        msk[0:64, cq * NH + 2 * cq] = 1.0
        msk[64:128, cq * NH + 2 * cq + 1] = 1.0

    if _NC_CACHE is None:
        _NC_CACHE = build_kernel()
    nc = _NC_CACHE

    in_maps = []
    for core in range(N_CORES):
        lo = core * DT
        # qT[p, (cq, d)] = q_scaled[b, lo + c*128 + d, cq*128 + p]
        qT = np.empty((BS, CHUNKS, 128, HW), dtype=np.float16)
        for b in range(BS):
            for c in range(CHUNKS):
                blk = qs[b, lo + c * 128 : lo + (c + 1) * 128]  # [128 d, 512]
                qT[b, c] = (
                    blk.reshape(128, CQ, 128).transpose(2, 1, 0).reshape(128, HW)
                )
        # edge i = s*128 + d; wrapped per 512-idx granule, concatenated
        idx = np.zeros((BS, CHUNKS, 128, S * 128 // 16), dtype=np.int16)
        for b in range(BS):
            for c in range(CHUNKS):
                flat = (
                    b * N_TOK + srct[lo + c * 128 : lo + (c + 1) * 128, :].T
                ).reshape(-1).astype(np.int16)  # [(s), (d)] flattened
                for g in range(KG):
                    idx[b, c, :, g * (KNI // 16) : (g + 1) * (KNI // 16)] = (
                        _wrap_idx(flat[g * KNI : (g + 1) * KNI])
                    )
        in_maps.append(
            {"kt": kt, "vt": vt, "qT": qT, "idx": idx, "msk": msk,
             "idn": np.eye(128, dtype=np.float16)}
        )

    res = run_bass_kernel_spmd(nc, in_maps, list(range(N_CORES)))
    out = np.empty((BS, N_TOK, NH, W), dtype=np.float32)
    for core in range(N_CORES):
        lo = core * DT
        o = res.results[core]["oc"].astype(np.float32)  # [BS, CHUNKS, 128, (w h)]
        o = o.reshape(BS, CHUNKS, 128, W, NH).transpose(0, 1, 2, 4, 3)
        out[:, lo : lo + DT] = o.reshape(BS, DT, NH, W)
    return out


# revision 17
# speedup vs baseline: 2.8296x; 1.0921x over previous
"""Sparse L1-distance attention (nn_L1AttnSparse) on 8 Trainium2 NeuronCores.

Sharding: dst tokens split across the 8 cores (256 dst each, x2 batches).
Per (batch, 128-dst chunk) iteration the 4096 edges (128 dst x 32 slots)
are processed as:

  - k rows gathered fp16 TRANSPOSED ([w-lane partitions, edge free], 8
    gathers of 512 edges) so the per-(slot,head) |q-k| sum over w runs on
    the PE array: stationary = |kT - qT| slot-plane [128 w-lanes, 128 dst],
    moving = an 8-col head mask; PSUM accumulates the 4 w-lane groups and
    scores land directly as [128 dst, (slot, head)].
  - softmax skips the max-subtraction (L = sum|q-k|/8 <= ~25 so exp(-L)
    stays inside fp32 range): just exp on ACT + sum + reciprocal.
  - v rows gathered fp16 untransposed in w-major row layout (4 gathers of
    1024 edges) so the weight broadcast keeps every operand innermost-
    packed and all elementwise work runs in the DVE 2x fp16 mode; the slot
    sum is an in-place pairwise tree.

k and q are pre-scaled by 1/8 (= 1/sqrt(64)) on the host.
"""

import sys

sys.path.insert(0, "/opt/trn_rl_repo")

import numpy as np

import concourse.bass as bass
import concourse.tile as tile
from concourse import bacc, mybir
from concourse.bass_utils import run_bass_kernel_spmd

BS = 2
N_TOK = 2048
NH = 8
W = 64
S = 32  # dst_mxlen
HW = NH * W  # 512 values per token row
N_CORES = 8
DT = N_TOK // N_CORES  # dst tokens per core = 256
CHUNKS = DT // 128  # dst chunks of 128 per core = 2
CQ = HW // 128  # 4 w-lane groups (transposed gather rows per partition)
KG = 8  # k gathers per chunk (512 edges each = 4 slots)
KS = S // KG  # 4 slots per k gather
KNI = 128 * KS  # 512
VG = 4  # v gathers per chunk (1024 edges each = 8 slots)
VS = S // VG  # 8 slots per v gather
VNI = 128 * VS  # 1024


def _wrap_idx(flat):
    """int16 index list -> [128, n/16] tile layout: idx i at [i%16, i//16],
    replicated down the 8 groups of 16 partitions."""
    n = flat.shape[0]
    w16 = np.zeros((16, n // 16), dtype=np.int16)
    w16[np.arange(n) % 16, np.arange(n) // 16] = flat
    return np.tile(w16, (8, 1))


def build_kernel():
    nc = bacc.Bacc(
        "TRN2", target_bir_lowering=False, debug=False, num_devices=N_CORES,
        dynamic_dma_scratch_size=16384 * 4,
    )
    f16 = mybir.dt.float16
    f32 = mybir.dt.float32
    i16 = mybir.dt.int16
    A = mybir.AluOpType

    kt = nc.dram_tensor("kt", [BS * N_TOK, HW], f16, kind="ExternalInput").ap()
    vt = nc.dram_tensor("vt", [BS * N_TOK, HW], f16, kind="ExternalInput").ap()
    qT = nc.dram_tensor("qT", [BS, CHUNKS, 128, HW], f16, kind="ExternalInput").ap()
    idx = nc.dram_tensor(
        "idx", [BS, CHUNKS, 128, S * 128 // 16], i16, kind="ExternalInput"
    ).ap()
    msk = nc.dram_tensor("msk", [128, CQ * NH], f16, kind="ExternalInput").ap()
    idn = nc.dram_tensor("idn", [128, 128], f16, kind="ExternalInput").ap()
    oc = nc.dram_tensor("oc", [BS, CHUNKS, 128, HW], f16, kind="ExternalOutput").ap()

    with tile.TileContext(nc) as tc:
        with (
            tc.tile_pool(name="kp", bufs=10) as kp,      # 0.5MB k gather tiles
            tc.tile_pool(name="vp", bufs=6) as vp,       # 1MB v gather tiles
            tc.tile_pool(name="sp", bufs=10) as sp,      # small tiles
            tc.tile_pool(name="ip", bufs=3) as ip,       # idx tiles
            tc.psum_pool(name="pp", bufs=2) as pp,       # scores
        ):
            msk_t = sp.tile([128, CQ * NH], f16, tag="msk", bufs=1)
            nc.sync.dma_start(out=msk_t[:], in_=msk)
            id_t = sp.tile([128, 128], f16, tag="idn", bufs=1)
            nc.sync.dma_start(out=id_t[:], in_=idn)

            for b in range(BS):
                for c in range(CHUNKS):
                    it = ip.tile([128, S * 128 // 16], i16, tag="idx")
                    nc.sync.dma_start(out=it[:], in_=idx[b, c])
                    itv = it[:].rearrange("p (g n) -> p g n", n=KNI // 16)
                    q_t = sp.tile([128, HW], f16, tag="qT")
                    nc.sync.dma_start(out=q_t[:], in_=qT[b, c])
                    qv = q_t[:].rearrange("p (cq d) -> p cq d", d=128)[
                        :, :, None, :
                    ].to_broadcast([128, CQ, KS, 128])

                    psum = pp.tile([128, S * NH], f32, tag="L")
                    E = sp.tile([128, S * NH], f16, tag="E")

                    def k_granule(g):
                        kg = kp.tile([128, CQ, KNI], f16, tag="kg")
                        nc.gpsimd.dma_gather(
                            kg[:], kt, itv[:, g], KNI, KNI, HW,
                            transpose=True, queue_num=0,
                        )
                        kv4 = kg[:].rearrange("p cq (s d) -> p cq s d", d=128)
                        # D = k/8 - q/8 (in place on the gather tile)
                        nc.vector.tensor_tensor(
                            out=kv4, in0=kv4, in1=qv, op=A.subtract,
                        )
                        # |D| in place on the ACT engine
                        nc.scalar.activation(
                            out=kg[:], in_=kg[:],
                            func=mybir.ActivationFunctionType.Abs,
                        )
                        # PE: L[d, (s h)] += sum_w |D| via head-mask matmuls
                        for s in range(KS):
                            sl = (g * KS + s) * NH
                            for cq in range(CQ):
                                nc.tensor.matmul(
                                    psum[:, sl : sl + NH],
                                    kv4[:, cq, s, :],
                                    msk_t[:, cq * NH : (cq + 1) * NH],
                                    start=(cq == 0),
                                    stop=(cq == CQ - 1),
                                )
                        # early per-granule exp so the v side can start
                        # before the remaining score granules finish
                        gs = g * KS * NH
                        nc.scalar.activation(
                            out=E[:, gs : gs + KS * NH],
                            in_=psum[:, gs : gs + KS * NH],
                            func=mybir.ActivationFunctionType.Exp, scale=-1.0,
                        )

                    psum_o = pp.tile([128, HW], f32, tag="O")

                    def v_granule(g):
                        vg = vp.tile([128, VS, HW], f16, tag="vg")
                        nc.gpsimd.dma_gather(
                            vg[:], vt,
                            it[:].rearrange("p (g n) -> p g n", n=VNI // 16)[:, g],
                            VNI, VNI, HW, queue_num=0,
                        )
                        ev = E[:, g * VS * NH : (g + 1) * VS * NH].rearrange(
                            "p (s h) -> p s h", h=NH
                        )[:, :, None, :].to_broadcast([128, VS, W, NH])
                        # weighted products in place
                        nc.vector.tensor_tensor(
                            out=vg[:].rearrange("p s (w h) -> p s w h", h=NH),
                            in0=vg[:].rearrange("p s (w h) -> p s w h", h=NH),
                            in1=ev,
                            op=A.mult,
                        )
                        # slot sum on PE: psum_o += I @ P_s
                        for s in range(VS):
                            nc.tensor.matmul(
                                psum_o[:],
                                id_t[:],
                                vg[:, s],
                                start=(g == 0 and s == 0),
                                stop=(g == VG - 1 and s == VS - 1),
                                skip_group_check=True,
                            )

                    # k gathers first (their compute chain is the long pole),
                    # then the v stream lands against ready score granules
                    for g in range(KG):
                        k_granule(g)
                    for g in range(VG):
                        v_granule(g)

                    # normalizer (needs all granules; applied at the end)
                    den = sp.tile([128, NH], f32, tag="den")
                    nc.vector.tensor_reduce(
                        out=den[:],
                        in_=E[:].rearrange("p (s h) -> p h s", h=NH),
                        axis=mybir.AxisListType.X,
                        op=A.add,
                    )
                    rden = sp.tile([128, NH], f32, tag="rden")
                    nc.vector.reciprocal(rden[:], den[:])
                    # normalize: out = psum_o * (1/den), broadcast over w
                    ot = sp.tile([128, HW], f16, tag="oc")
                    nc.vector.tensor_tensor(
                        out=ot[:].rearrange("p (w h) -> p w h", h=NH),
                        in0=psum_o[:].rearrange("p (w h) -> p w h", h=NH),
                        in1=rden[:][:, None, :].to_broadcast([128, W, NH]),
                        op=A.mult,
                    )
                    nc.sync.dma_start(out=oc[b, c], in_=ot[:])
    nc.compile()
    return nc


_NC_CACHE = None


def kernel(v, q, k, coo, dst_mxlen):
    global _NC_CACHE
    assert int(dst_mxlen) == S
    v = np.asarray(v, dtype=np.float32)
    q = np.asarray(q, dtype=np.float32)
    k = np.asarray(k, dtype=np.float32)
    coo = np.asarray(coo)

    # src table: srct[t, s] = src index of edge (dst=t, slot=s)
    srct = np.zeros((N_TOK, S), dtype=np.int64)
    srct[coo[:, 0], coo[:, 2]] = coo[:, 1]

    scale = 1.0 / np.sqrt(W)
    kt = (k * scale).astype(np.float16).reshape(BS * N_TOK, HW)
    # v table in w-major row layout: row[(w, h)] = v[h, w]
    vt = np.ascontiguousarray(v.transpose(0, 1, 3, 2)).astype(np.float16)
    vt = vt.reshape(BS * N_TOK, HW)
    qs = (q * scale).astype(np.float16).reshape(BS, N_TOK, HW)

    # PE head masks: msk[p, cq*8 + h] = 1 iff h == 2*cq + (p >= 64)
    msk = np.zeros((128, CQ * NH), dtype=np.float16)
    for cq in range(CQ):
        msk[0:64, cq * NH + 2 * cq] = 1.0
        msk[64:128, cq * NH + 2 * cq + 1] = 1.0

    if _NC_CACHE is None:
        _NC_CACHE = build_kernel()
    nc = _NC_CACHE

    in_maps = []
    for core in range(N_CORES):
        lo = core * DT
        # qT[p, (cq, d)] = q_scaled[b, lo + c*128 + d, cq*128 + p]
        qT = np.empty((BS, CHUNKS, 128, HW), dtype=np.float16)
        for b in range(BS):
            for c in range(CHUNKS):
                blk = qs[b, lo + c * 128 : lo + (c + 1) * 128]  # [128 d, 512]
                qT[b, c] = (
                    blk.reshape(128, CQ, 128).transpose(2, 1, 0).reshape(128, HW)
                )
        # edge i = s*128 + d; wrapped per 512-idx granule, concatenated
        idx = np.zeros((BS, CHUNKS, 128, S * 128 // 16), dtype=np.int16)
        for b in range(BS):
            for c in range(CHUNKS):
                flat = (
                    b * N_TOK + srct[lo + c * 128 : lo + (c + 1) * 128, :].T
                ).reshape(-1).astype(np.int16)  # [(s), (d)] flattened
                for g in range(KG):
                    idx[b, c, :, g * (KNI // 16) : (g + 1) * (KNI // 16)] = (
                        _wrap_idx(flat[g * KNI : (g + 1) * KNI])
                    )
        in_maps.append(
            {"kt": kt, "vt": vt, "qT": qT, "idx": idx, "msk": msk,
             "idn": np.eye(128, dtype=np.float16)}
        )

    res = run_bass_kernel_spmd(nc, in_maps, list(range(N_CORES)))
    out = np.empty((BS, N_TOK, NH, W), dtype=np.float32)
    for core in range(N_CORES):
        lo = core * DT
        o = res.results[core]["oc"].astype(np.float32)  # [BS, CHUNKS, 128, (w h)]
        o = o.reshape(BS, CHUNKS, 128, W, NH).transpose(0, 1, 2, 4, 3)
        out[:, lo : lo + DT] = o.reshape(BS, DT, NH, W)
    return out


# revision 19
# speedup vs baseline: 2.8867x; 1.0202x over previous
"""Sparse L1-distance attention (nn_L1AttnSparse) on 8 Trainium2 NeuronCores.

Sharding: dst tokens split across the 8 cores (256 dst each, x2 batches).
Per (batch, 128-dst chunk) iteration the 4096 edges (128 dst x 32 slots)
are processed as:

  - k rows gathered fp16 TRANSPOSED ([w-lane partitions, edge free], 8
    gathers of 512 edges) so the per-(slot,head) |q-k| sum over w runs on
    the PE array: stationary = |kT - qT| slot-plane [128 w-lanes, 128 dst],
    moving = an 8-col head mask; PSUM accumulates the 4 w-lane groups and
    scores land directly as [128 dst, (slot, head)].
  - softmax skips the max-subtraction (L = sum|q-k|/8 <= ~25 so exp(-L)
    stays inside fp32 range): just exp on ACT + sum + reciprocal.
  - v rows gathered fp16 untransposed in w-major row layout (4 gathers of
    1024 edges) so the weight broadcast keeps every operand innermost-
    packed and all elementwise work runs in the DVE 2x fp16 mode; the slot
    sum is an in-place pairwise tree.

k and q are pre-scaled by 1/8 (= 1/sqrt(64)) on the host.
"""

import sys

sys.path.insert(0, "/opt/trn_rl_repo")

import numpy as np

import concourse.bass as bass
import concourse.tile as tile
from concourse import bacc, mybir
from concourse.bass_utils import run_bass_kernel_spmd

BS = 2
N_TOK = 2048
NH = 8
W = 64
S = 32  # dst_mxlen
HW = NH * W  # 512 values per token row
N_CORES = 8
DT = N_TOK // N_CORES  # dst tokens per core = 256
CHUNKS = DT // 128  # dst chunks of 128 per core = 2
CQ = HW // 128  # 4 w-lane groups (transposed gather rows per partition)
KG = 8  # k gathers per chunk (512 edges each = 4 slots)
KS = S // KG  # 4 slots per k gather
KNI = 128 * KS  # 512
VG = 4  # v gathers per chunk (1024 edges each = 8 slots)
VS = S // VG  # 8 slots per v gather
VNI = 128 * VS  # 1024


def _wrap_idx(flat):
    """int16 index list -> [128, n/16] tile layout: idx i at [i%16, i//16],
    replicated down the 8 groups of 16 partitions."""
    n = flat.shape[0]
    w16 = np.zeros((16, n // 16), dtype=np.int16)
    w16[np.arange(n) % 16, np.arange(n) // 16] = flat
    return np.tile(w16, (8, 1))


def build_kernel():
    nc = bacc.Bacc(
        "TRN2", target_bir_lowering=False, debug=False, num_devices=N_CORES,
        dynamic_dma_scratch_size=16384 * 4,
    )
    f16 = mybir.dt.float16
    f32 = mybir.dt.float32
    i16 = mybir.dt.int16
    A = mybir.AluOpType

    kt = nc.dram_tensor("kt", [BS * N_TOK, HW], f16, kind="ExternalInput").ap()
    vt = nc.dram_tensor("vt", [BS * N_TOK, HW], f16, kind="ExternalInput").ap()
    qT = nc.dram_tensor("qT", [BS, CHUNKS, 128, HW], f16, kind="ExternalInput").ap()
    idx = nc.dram_tensor(
        "idx", [BS, CHUNKS, 128, S * 128 // 16], i16, kind="ExternalInput"
    ).ap()
    msk = nc.dram_tensor("msk", [128, CQ * NH], f16, kind="ExternalInput").ap()
    idn = nc.dram_tensor("idn", [128, 128], f16, kind="ExternalInput").ap()
    oc = nc.dram_tensor("oc", [BS, CHUNKS, 128, HW], f16, kind="ExternalOutput").ap()

    with tile.TileContext(nc) as tc:
        with (
            tc.tile_pool(name="kp", bufs=10) as kp,      # 0.5MB k gather tiles
            tc.tile_pool(name="vp", bufs=6) as vp,       # 1MB v gather tiles
            tc.tile_pool(name="sp", bufs=10) as sp,      # small tiles
            tc.tile_pool(name="ip", bufs=3) as ip,       # idx tiles
            tc.psum_pool(name="pp", bufs=2) as pp,       # scores
        ):
            NIT = BS * CHUNKS
            it_all = ip.tile([128, NIT, S * 128 // 16], i16, tag="idx", bufs=1)
            nc.sync.dma_start(
                out=it_all[:], in_=idx.rearrange("b c p n -> p (b c) n")
            )
            q_all = sp.tile([128, NIT, HW], f16, tag="qT", bufs=1)
            nc.sync.dma_start(
                out=q_all[:], in_=qT.rearrange("b c p n -> p (b c) n")
            )
            msk_t = sp.tile([128, CQ * NH], f16, tag="msk", bufs=1)
            nc.sync.dma_start(out=msk_t[:], in_=msk)
            id_t = sp.tile([128, 128], f16, tag="idn", bufs=1)
            nc.sync.dma_start(out=id_t[:], in_=idn)

            for b in range(BS):
                for c in range(CHUNKS):
                    bc = b * CHUNKS + c
                    it = it_all[:, bc]
                    itv = it_all[:, bc].rearrange("p (g n) -> p g n", n=KNI // 16)
                    qv = q_all[:, bc].rearrange("p (cq d) -> p cq d", d=128)[
                        :, :, None, :
                    ].to_broadcast([128, CQ, KS, 128])

                    psum = pp.tile([128, S * NH], f32, tag="L")
                    E = sp.tile([128, S * NH], f16, tag="E")

                    def k_granule(g):
                        kg = kp.tile([128, CQ, KNI], f16, tag="kg")
                        nc.gpsimd.dma_gather(
                            kg[:], kt, itv[:, g], KNI, KNI, HW,
                            transpose=True, queue_num=0,
                        )
                        kv4 = kg[:].rearrange("p cq (s d) -> p cq s d", d=128)
                        # D = k/8 - q/8 (in place on the gather tile)
                        nc.vector.tensor_tensor(
                            out=kv4, in0=kv4, in1=qv, op=A.subtract,
                        )
                        # |D| in place on the ACT engine
                        nc.scalar.activation(
                            out=kg[:], in_=kg[:],
                            func=mybir.ActivationFunctionType.Abs,
                        )
                        # PE: L[d, (s h)] += sum_w |D| via head-mask matmuls
                        for s in range(KS):
                            sl = (g * KS + s) * NH
                            for cq in range(CQ):
                                nc.tensor.matmul(
                                    psum[:, sl : sl + NH],
                                    kv4[:, cq, s, :],
                                    msk_t[:, cq * NH : (cq + 1) * NH],
                                    start=(cq == 0),
                                    stop=(cq == CQ - 1),
                                )
                        # early per-granule exp so the v side can start
                        # before the remaining score granules finish
                        gs = g * KS * NH
                        nc.scalar.activation(
                            out=E[:, gs : gs + KS * NH],
                            in_=psum[:, gs : gs + KS * NH],
                            func=mybir.ActivationFunctionType.Exp, scale=-1.0,
                        )

                    psum_o = pp.tile([128, HW], f32, tag="O")

                    def v_granule(s0, ns):
                        # gathers slots [s0, s0+ns), ns*128 indices
                        vg = vp.tile([128, ns, HW], f16, tag=f"vg{ns}")
                        nc.gpsimd.dma_gather(
                            vg[:], vt,
                            it_all[:, bc].rearrange(
                                "p (s n) -> p s n", n=128 // 16
                            )[:, s0 : s0 + ns].rearrange("p s n -> p (s n)"),
                            ns * 128, ns * 128, HW, queue_num=0,
                        )
                        ev = E[:, s0 * NH : (s0 + ns) * NH].rearrange(
                            "p (s h) -> p s h", h=NH
                        )[:, :, None, :].to_broadcast([128, ns, W, NH])
                        # weighted products in place
                        nc.vector.tensor_tensor(
                            out=vg[:].rearrange("p s (w h) -> p s w h", h=NH),
                            in0=vg[:].rearrange("p s (w h) -> p s w h", h=NH),
                            in1=ev,
                            op=A.mult,
                        )
                        # slot sum on PE: psum_o += I @ P_s
                        for s in range(ns):
                            nc.tensor.matmul(
                                psum_o[:],
                                id_t[:],
                                vg[:, s],
                                start=(s0 + s == 0),
                                stop=(s0 + s == S - 1),
                                skip_group_check=True,
                            )

                    # k gathers first (their compute chain is the long pole),
                    # then the v stream lands against ready score granules
                    for g in range(KG):
                        k_granule(g)
                    last = b == BS - 1 and c == CHUNKS - 1
                    for g in range(VG):
                        if last and g == VG - 1:
                            # split the final granule to shorten the tail
                            v_granule(g * VS, VS // 2)
                            v_granule(g * VS + VS // 2, VS // 2)
                        else:
                            v_granule(g * VS, VS)

                    # normalizer (needs all granules; applied at the end)
                    den = sp.tile([128, NH], f32, tag="den")
                    nc.vector.tensor_reduce(
                        out=den[:],
                        in_=E[:].rearrange("p (s h) -> p h s", h=NH),
                        axis=mybir.AxisListType.X,
                        op=A.add,
                    )
                    rden = sp.tile([128, NH], f32, tag="rden")
                    nc.vector.reciprocal(rden[:], den[:])
                    # normalize: out = psum_o * (1/den), broadcast over w
                    ot = sp.tile([128, HW], f16, tag="oc")
                    nc.vector.tensor_tensor(
                        out=ot[:].rearrange("p (w h) -> p w h", h=NH),
                        in0=psum_o[:].rearrange("p (w h) -> p w h", h=NH),
                        in1=rden[:][:, None, :].to_broadcast([128, W, NH]),
                        op=A.mult,
                    )
                    nc.sync.dma_start(out=oc[b, c], in_=ot[:])
    nc.compile()
    return nc


_NC_CACHE = None


def kernel(v, q, k, coo, dst_mxlen):
    global _NC_CACHE
    assert int(dst_mxlen) == S
    v = np.asarray(v, dtype=np.float32)
    q = np.asarray(q, dtype=np.float32)
    k = np.asarray(k, dtype=np.float32)
    coo = np.asarray(coo)

    # src table: srct[t, s] = src index of edge (dst=t, slot=s)
    srct = np.zeros((N_TOK, S), dtype=np.int64)
    srct[coo[:, 0], coo[:, 2]] = coo[:, 1]

    scale = 1.0 / np.sqrt(W)
    kt = (k * scale).astype(np.float16).reshape(BS * N_TOK, HW)
    # v table in w-major row layout: row[(w, h)] = v[h, w]
    vt = np.ascontiguousarray(v.transpose(0, 1, 3, 2)).astype(np.float16)
    vt = vt.reshape(BS * N_TOK, HW)
    qs = (q * scale).astype(np.float16).reshape(BS, N_TOK, HW)

    # PE head masks: msk[p, cq*8 + h] = 1 iff h == 2*cq + (p >= 64)
    msk = np.zeros((128, CQ * NH), dtype=np.float16)
    for cq in range(CQ):
        msk[0:64, cq * NH + 2 * cq] = 1.0
        msk[64:128, cq * NH + 2 * cq + 1] = 1.0

    if _NC_CACHE is None:
        _NC_CACHE = build_kernel()
    nc = _NC_CACHE

    in_maps = []
    for core in range(N_CORES):
        lo = core * DT
        # qT[p, (cq, d)] = q_scaled[b, lo + c*128 + d, cq*128 + p]
        qT = np.empty((BS, CHUNKS, 128, HW), dtype=np.float16)
        for b in range(BS):
            for c in range(CHUNKS):
                blk = qs[b, lo + c * 128 : lo + (c + 1) * 128]  # [128 d, 512]
                qT[b, c] = (
                    blk.reshape(128, CQ, 128).transpose(2, 1, 0).reshape(128, HW)
                )
        # edge i = s*128 + d; wrapped per 512-idx granule, concatenated
        idx = np.zeros((BS, CHUNKS, 128, S * 128 // 16), dtype=np.int16)
        for b in range(BS):
            for c in range(CHUNKS):
                flat = (
                    b * N_TOK + srct[lo + c * 128 : lo + (c + 1) * 128, :].T
                ).reshape(-1).astype(np.int16)  # [(s), (d)] flattened
                for g in range(KG):
                    idx[b, c, :, g * (KNI // 16) : (g + 1) * (KNI // 16)] = (
                        _wrap_idx(flat[g * KNI : (g + 1) * KNI])
                    )
        in_maps.append(
            {"kt": kt, "vt": vt, "qT": qT, "idx": idx, "msk": msk,
             "idn": np.eye(128, dtype=np.float16)}
        )

    res = run_bass_kernel_spmd(nc, in_maps, list(range(N_CORES)))
    out = np.empty((BS, N_TOK, NH, W), dtype=np.float32)
    for core in range(N_CORES):
        lo = core * DT
        o = res.results[core]["oc"].astype(np.float32)  # [BS, CHUNKS, 128, (w h)]
        o = o.reshape(BS, CHUNKS, 128, W, NH).transpose(0, 1, 2, 4, 3)
        out[:, lo : lo + DT] = o.reshape(BS, DT, NH, W)
    return out
